# revision 1
# baseline (speedup 1.0000x reference)
"""Trainium2 Bass kernel for DenseKANRBF.

Computation (per reference):
    centers c_g = linspace(-1, 1, 8)  (same for every feature)
    basis[b,f,g] = exp(-(x[b,f] - c_g)^2)
    out = einsum('bfg,fgu->bu', basis, basis_kernel)
        + gelu(x @ w1 + b1, exact) @ w2 + b2 + bias

Shapes: B=1024, F=512, G=8, U=512, H=2048 (fp32).

Strategy: data-parallel over batch across 8 NeuronCores (128 rows/core),
weights replicated and pre-cast to bf16 on host.  All matmuls bf16 with
fp32 PSUM accumulation.  Per core (DMA-roofline ~8.6MB @ ~360GB/s):

  - The uniform grid makes the RBF basis a geometric sequence:
        basis_g = exp(-(y - 2g/7)^2) = K_g * A * r^g,
        y = x+1, A = exp(-y^2), r = exp(4y/7), K_g = exp(-(2g/7)^2)
    K_g is folded into basis_kernel on the host.  A and r are computed
    in the *transposed packed* layout (xt4[p, j*128+b] = x[b, j*128+p]),
    so seven wide fp32 DVE multiplies + bf16 casts produce the basis
    already transposed for the PE - no on-chip transposes at all.
  - MLP1 runs weight-stationary (lhsT = w1 chunk), producing h.T tiles
    in PSUM; gelu reads them with a per-partition b1 bias fused into the
    ACT instruction, writing bf16 h.T tiles that feed MLP2 directly.
  - A run of dummy matmuls at kernel start holds the PE HAM clock at
    2.4 GHz so the real matmuls run warm.
  - One PSUM bank accumulates KAN + MLP2 + (b2+bias); DMA arrival order
    (w1, kg0..3, w2) matches the accumulation chain so only ~8 matmuls
    trail the last DMA byte.
"""

import os
from contextlib import ExitStack

import numpy as np
import ml_dtypes

import concourse.bass as bass
import concourse.bacc as bacc
import concourse.mybir as mybir
from concourse import tile
from concourse.bass_utils import run_bass_kernel_spmd

F32 = mybir.dt.float32
BF16 = mybir.dt.bfloat16
AF = mybir.ActivationFunctionType

B, F, G, U, H = 1024, 512, 8, 512, 2048
NCORES = 8
BL = B // NCORES  # 128 rows per core
NWARM = 30  # PE HAM warm-up matmuls

bf16 = ml_dtypes.bfloat16

_prog_cache = None

# xt4 layout: [:, :512] = x.T packed fp32; then consts and b1.T columns
XC_ONE = F  # +1.0
XC_NEG1 = F + 1  # -1.0
XC_R = F + 2  # 4/7
XC_B1 = F + 3  # b1T[p, k] = b1[k*128+p], 16 cols
XT4_W = F + 3 + 16


def _build_program():
    nc = bacc.Bacc("TRN2", target_bir_lowering=False, debug=False, num_devices=NCORES)

    xt4_d = nc.dram_tensor("xt4", [128, XT4_W], F32, kind="ExternalInput")
    # vecs: [0:512]=b2+bias, [512:640]=ones
    vecs_d = nc.dram_tensor("vecs", [1, U + 128], BF16, kind="ExternalInput")
    # w1 packed [128, 4*H]: w1p[p, l*H + h] = w1[l*128 + p, h]
    w1_d = nc.dram_tensor("w1", [128, 4 * H], BF16, kind="ExternalInput")
    # basis_kernel g-major, K_g-scaled, split 16/8/4/4 h-chunks of 128 rows
    kga_d = nc.dram_tensor("kga", [128, 16 * U], BF16, kind="ExternalInput")
    kgb_d = nc.dram_tensor("kgb", [128, 8 * U], BF16, kind="ExternalInput")
    kgc_d = nc.dram_tensor("kgc", [128, 4 * U], BF16, kind="ExternalInput")
    kgd_d = nc.dram_tensor("kgd", [128, 4 * U], BF16, kind="ExternalInput")
    # w2 packed: w2a rows 0..11, w2b rows 12..15 (h-chunks of 128)
    w2a_d = nc.dram_tensor("w2a", [128, 12 * U], BF16, kind="ExternalInput")
    w2b_d = nc.dram_tensor("w2b", [128, 4 * U], BF16, kind="ExternalInput")
    out_d = nc.dram_tensor("out", [BL, U], F32, kind="ExternalOutput")

    with ExitStack() as ctx:
        tc = ctx.enter_context(tile.TileContext(nc))
        const = ctx.enter_context(tc.tile_pool(name="const", bufs=1))
        chain = ctx.enter_context(tc.tile_pool(name="chain", bufs=2))
        btp = ctx.enter_context(tc.tile_pool(name="btp", bufs=8))
        htp = ctx.enter_context(tc.tile_pool(name="htp", bufs=16))
        hps_pool = ctx.enter_context(
            tc.tile_pool(name="hps", bufs=6, space=bass.MemorySpace.PSUM)
        )
        wps_pool = ctx.enter_context(
            tc.tile_pool(name="wps", bufs=1, space=bass.MemorySpace.PSUM)
        )
        ops_pool = ctx.enter_context(
            tc.tile_pool(name="ops", bufs=1, space=bass.MemorySpace.PSUM)
        )

        # ---- ACT exp-table preload + PE HAM warm-up (no input deps) ----
        warm = const.tile([128, 1], F32, tag="warm")
        nc.gpsimd.memset(warm[:], 0.0)
        nc.scalar.activation(warm[:], warm[:], AF.Exp)
        wl = const.tile([128, 128], BF16, tag="wl")
        nc.gpsimd.memset(wl[:], 0.0)
        wr = const.tile([128, 512], BF16, tag="wr")
        nc.gpsimd.memset(wr[:], 0.0)
        wps = wps_pool.tile([128, 512], F32)
        for _ in range(NWARM):
            nc.tensor.matmul(wps[:], wl[:], wr[:], start=True, stop=True)

        # ---- loads (nc.sync HWDGE => FIFO in emission order) ----
        xt4_sb = const.tile([128, XT4_W], F32, tag="xt4")
        nc.sync.dma_start(xt4_sb[:], xt4_d[:])
        vecs_sb = const.tile([1, U + 128], BF16, tag="vecs")
        nc.sync.dma_start(vecs_sb[:], vecs_d[:])
        w1_sb = const.tile([128, 4 * H], BF16, tag="w1")
        nc.sync.dma_start(w1_sb[:], w1_d[:])
        w2a_sb = const.tile([128, 12 * U], BF16, tag="w2a")
        nc.sync.dma_start(w2a_sb[:], w2a_d[:])
        w2b_sb = const.tile([128, 4 * U], BF16, tag="w2b")
        nc.sync.dma_start(w2b_sb[:], w2b_d[:])
        kga_sb = const.tile([128, 16 * U], BF16, tag="kga")
        nc.sync.dma_start(kga_sb[:], kga_d[:])
        kgb_sb = const.tile([128, 8 * U], BF16, tag="kgb")
        nc.sync.dma_start(kgb_sb[:], kgb_d[:])
        kgc_sb = const.tile([128, 4 * U], BF16, tag="kgc")
        nc.sync.dma_start(kgc_sb[:], kgc_d[:])
        kgd_sb = const.tile([128, 4 * U], BF16, tag="kgd")
        nc.sync.dma_start(kgd_sb[:], kgd_d[:])
        kg_parts = [(kga_sb, 0, 16), (kgb_sb, 16, 8), (kgc_sb, 24, 4), (kgd_sb, 28, 4)]

        xt_f32 = xt4_sb[:, 0:F]
        one_c = xt4_sb[:, XC_ONE : XC_ONE + 1]
        neg1_c = xt4_sb[:, XC_NEG1 : XC_NEG1 + 1]
        r_c = xt4_sb[:, XC_R : XC_R + 1]
        b1T = lambda k: xt4_sb[:, XC_B1 + k : XC_B1 + k + 1]
        bcv = vecs_sb[0:1, 0:U]
        ones = vecs_sb[0:1, U : U + 128]

        def w1_blk(kc, k):  # [128 f, 128 h]: f rows kc*128.., h cols k*128..
            return w1_sb[:, kc * H + k * 128 : kc * H + (k + 1) * 128]

        def w2_chunk(k):  # [128, 512] for h rows k*128..
            if k < 12:
                return w2a_sb[:, k * U : (k + 1) * U]
            return w2b_sb[:, (k - 12) * U : (k - 11) * U]

        def kg_chunk(i):  # [128, 512] rows i*128.. of g-major (4096, 512)
            for t, base, n in kg_parts:
                if base <= i < base + n:
                    return t[:, (i - base) * U : (i - base + 1) * U]
            raise AssertionError(i)

        # ---- bf16 x.T for MLP1 rhs ----
        xt_bf = const.tile([128, F], BF16, tag="xtbf")
        nc.vector.tensor_copy(xt_bf[:], xt_f32)

        # ---- basis chain in transposed layout ----
        y = const.tile([128, F], F32, tag="y")
        nc.vector.tensor_scalar_add(y[:], xt_f32, one_c)
        s = const.tile([128, F], F32, tag="s")
        nc.vector.tensor_mul(s[:], y[:], y[:])
        r = const.tile([128, F], F32, tag="r")
        nc.scalar.activation(r[:], y[:], AF.Exp, scale=r_c)
        t_prev = chain.tile([128, F], F32, tag="t")
        nc.scalar.activation(t_prev[:], s[:], AF.Exp, scale=neg1_c)  # A

        bt = []  # bf16 basis tiles, transposed layout, per g
        for g in range(G):
            if g > 0:
                t_cur = chain.tile([128, F], F32, tag="t")
                nc.vector.tensor_mul(t_cur[:], t_prev[:], r[:])
                t_prev = t_cur
            c = btp.tile([128, F], BF16, tag="bt")
            nc.vector.tensor_copy(c[:], t_prev[:])
            bt.append(c)

        # ---- MLP1 weight-stationary: hT psum tiles + fused-bias gelu ----
        gelu_fn = AF.Identity if os.environ.get("TRN_SIM_NOGELU") else AF.Gelu
        ht = []
        for k in range(16):
            hps = hps_pool.tile([128, BL], F32)
            for kc in range(4):
                nc.tensor.matmul(
                    hps[:],
                    w1_blk(kc, k),
                    xt_bf[:, kc * BL : (kc + 1) * BL],
                    start=(kc == 0),
                    stop=(kc == 3),
                )
            t = htp.tile([128, BL], BF16, tag="ht")
            nc.scalar.activation(t[:], hps[:], gelu_fn, bias=b1T(k))
            ht.append(t)

        # ---- accumulation bank: (b2+bias) -> MLP2 -> KAN ----
        out_ps = ops_pool.tile([BL, U], F32)
        nc.tensor.matmul(
            out_ps[:], ones, bcv, start=True, stop=False, skip_group_check=True
        )
        for k in range(16):
            nc.tensor.matmul(
                out_ps[:],
                ht[k][:],
                w2_chunk(k),
                start=False,
                stop=False,
                skip_group_check=True,
            )
        for i in range(32):
            g, fc = divmod(i, 4)
            nc.tensor.matmul(
                out_ps[:],
                bt[g][:, fc * 128 : (fc + 1) * 128],
                kg_chunk(i),
                start=False,
                stop=(i == 31),
                skip_group_check=True,
            )

        out_sb = const.tile([BL, U], F32, tag="outsb")
        nc.vector.tensor_copy(out_sb[:], out_ps[:])
        nc.sync.dma_start(out_d[:], out_sb[:])

    nc.compile()
    return nc


def _host_prep(x, basis_kernel, mlp_w1, mlp_b1, mlp_w2, mlp_b2, bias):
    """Shared (per-core-independent) input packing."""
    w1p = (
        mlp_w1.reshape(4, 128, H).transpose(1, 0, 2).reshape(128, 4 * H).astype(bf16)
    )
    w2r = mlp_w2.reshape(16, 128, U)
    w2pa = (
        w2r[:12].transpose(1, 0, 2).reshape(128, 12 * U).astype(bf16)
    )
    w2pb = (
        w2r[12:].transpose(1, 0, 2).reshape(128, 4 * U).astype(bf16)
    )
    # g-major with K_g = exp(-(2g/7)^2) folded in
    gidx = np.arange(G, dtype=np.float64)
    kscale = np.exp(-((2.0 * gidx / 7.0) ** 2)).astype(np.float32)
    kgf = (basis_kernel.transpose(1, 0, 2) * kscale[:, None, None]).reshape(
        G * F, U
    )
    kgr = kgf.reshape(32, 128, U)
    kga = kgr[0:16].transpose(1, 0, 2).reshape(128, 16 * U).astype(bf16)
    kgb = kgr[16:24].transpose(1, 0, 2).reshape(128, 8 * U).astype(bf16)
    kgc = kgr[24:28].transpose(1, 0, 2).reshape(128, 4 * U).astype(bf16)
    kgd = kgr[28:32].transpose(1, 0, 2).reshape(128, 4 * U).astype(bf16)
    vecs = np.zeros((1, U + 128), bf16)
    vecs[0, :U] = (mlp_b2 + bias).astype(bf16)
    vecs[0, U:] = np.ones(128, bf16)
    b1t = np.ascontiguousarray(mlp_b1.reshape(16, 128).T).astype(np.float32)
    return {
        "vecs": vecs,
        "w1": w1p,
        "w2a": w2pa,
        "w2b": w2pb,
        "kga": kga,
        "kgb": kgb,
        "kgc": kgc,
        "kgd": kgd,
        "_b1t": b1t,
    }


def kernel(x, basis_kernel, mlp_w1, mlp_b1, mlp_w2, mlp_b2, bias):
    global _prog_cache
    x = np.asarray(x, dtype=np.float32)
    common = _host_prep(
        x,
        np.asarray(basis_kernel, dtype=np.float32),
        np.asarray(mlp_w1, dtype=np.float32),
        np.asarray(mlp_b1, dtype=np.float32),
        np.asarray(mlp_w2, dtype=np.float32),
        np.asarray(mlp_b2, dtype=np.float32),
        np.asarray(bias, dtype=np.float32),
    )
    b1t = common.pop("_b1t")

    in_maps = []
    for c in range(NCORES):
        xrows = x[c * BL : (c + 1) * BL]  # [128, 512]
        xt4 = np.zeros((128, XT4_W), np.float32)
        xt4[:, :F] = xrows.reshape(BL, 4, 128).transpose(2, 1, 0).reshape(128, F)
        xt4[:, XC_ONE] = 1.0
        xt4[:, XC_NEG1] = -1.0
        xt4[:, XC_R] = 4.0 / 7.0
        xt4[:, XC_B1 : XC_B1 + 16] = b1t
        in_maps.append({"xt4": xt4, **common})

    if _prog_cache is None:
        _prog_cache = _build_program()
    nc = _prog_cache

    trace = bool(int(os.environ.get("TRN_KERNEL_TRACE", "0")))
    if trace:
        _install_profile_hook()
    res = run_bass_kernel_spmd(
        nc,
        in_maps,
        core_ids=list(range(NCORES)),
        trace=trace,
    )
    if trace:
        print(f"HW exec time: {res.exec_time_ns} ns")
        kernel.last_results = res

    out = np.concatenate([res.results[c]["out"] for c in range(NCORES)], axis=0)
    return out.astype(np.float32)


kernel.last_results = None


def _install_profile_hook():
    """The image lacks antenv.axon_hooks; synthesize it so
    run_bass_kernel_spmd(trace=True) can reach the NTFF profiler in
    libaxon_pjrt.so.  Test-only path (TRN_KERNEL_TRACE=1)."""
    import sys
    import types

    if "antenv.axon_hooks" not in sys.modules:
        mod = types.ModuleType("antenv.axon_hooks")
        mod._hook = None

        def set_axon_ntff_profile_hook(h):
            mod._hook = h

        def get_axon_ntff_profile_hook():
            return mod._hook

        mod.set_axon_ntff_profile_hook = set_axon_ntff_profile_hook
        mod.get_axon_ntff_profile_hook = get_axon_ntff_profile_hook
        sys.modules["antenv.axon_hooks"] = mod
        import antenv

        antenv.axon_hooks = mod
        from trn_agent_boot.trn_boot import _ntff_profile_via_ctypes

        mod.set_axon_ntff_profile_hook(
            _ntff_profile_via_ctypes("/opt/axon/libaxon_pjrt.so")
        )
    import concourse.bass_utils as _bu

    _bu.upload_artifacts = lambda tmpdir: f"local:{tmpdir}"



# revision 2
# speedup vs baseline: 1.0512x; 1.0512x over previous
"""Trainium2 Bass kernel for DenseKANRBF.

Computation (per reference):
    centers c_g = linspace(-1, 1, 8)  (same for every feature)
    basis[b,f,g] = exp(-(x[b,f] - c_g)^2)
    out = einsum('bfg,fgu->bu', basis, basis_kernel)
        + gelu(x @ w1 + b1, exact) @ w2 + b2 + bias

Shapes: B=1024, F=512, G=8, U=512, H=2048 (fp32).

Strategy: HYBRID sharding over the 8 NeuronCores: 4 batch shards x 2
U-shards (rb=4, ru=2).  Each core handles 256 batch rows and 256 output
columns, so per-core DMA drops from 8.6MB (pure data-parallel,
replicated weights) to ~5.8MB, while PE work stays ~17us -- balanced
against the ~16us DMA at 360GB/s.  No cross-core communication; the
host scatters shards and gathers the 4x2 output grid.

Key device-side choices (from trace/cost-model analysis of the 41us
data-parallel baseline, which was pinned at the replicated-weight DMA
floor with a 3.4us PE stall + clock droop before the KAN tail):

  - The uniform grid makes the RBF basis a geometric sequence:
        basis_g = K_g * t0 * r^g,  t0 = exp(-(x+1)^2), r = exp(4(x+1)/7)
    K_g is folded into basis_kernel on the host.  t0 and r are ALSO
    computed on the host (f64 -> bf16, +0.5MB DMA) so the device does
    only 7 bf16 DVE multiplies for the whole basis -- no Exp
    activations at all.  That leaves Gelu as the only ACT table
    (loaded once via a dummy at t=0) and frees Scalar to stream gelus
    as soon as MLP1 PSUM tiles appear, which keeps the 6-buffer PSUM
    recycling stall-free.
  - All tensors are pre-packed/transposed on the host so every matmul
    operand is a contiguous [128, N] slice: xt/basis in f-partition
    packed layout, w1/w2/kg in contraction-partition packed layout.
  - Weight DMAs are split into 512KB chunks (w1 x4, w2 x2, kg x4) so
    matmuls track chunk arrival instead of whole-tensor completion.
  - PE order: warmup (clock ramp) -> MLP1 (64 mm) -> bias+MLP2 (34 mm)
    -> KAN (64 mm), with both batch halves accumulating in disjoint
    column ranges of ONE PSUM bank; half-0's chain stops 8 matmuls
    early so its copy+store overlaps half-1's tail.
"""

import os
from contextlib import ExitStack

import numpy as np
import ml_dtypes

import concourse.bass as bass
import concourse.bacc as bacc
import concourse.mybir as mybir
from concourse import tile
from concourse.bass_utils import run_bass_kernel_spmd

F32 = mybir.dt.float32
BF16 = mybir.dt.bfloat16
AF = mybir.ActivationFunctionType

B, F, G, U, H = 1024, 512, 8, 512, 2048
NCORES = 8
RB, RU = 4, 2  # batch shards x U shards
BL = B // RB  # 256 batch rows per core
UL = U // RU  # 256 output cols per core
NWARM = 16  # PE clock-ramp warm-up matmuls

bf16 = ml_dtypes.bfloat16

_prog_cache = None


def _build_program():
    nc = bacc.Bacc("TRN2", target_bir_lowering=False, debug=False, num_devices=NCORES)

    # xt[p, j*256+b] = x[s*256+b, j*128+p]   (f-chunk j, batch b)
    xt_d = nc.dram_tensor("xt", [128, 4 * BL], BF16, kind="ExternalInput")
    # rt = [t0 | r] in the same packed layout
    rt_d = nc.dram_tensor("rt", [128, 8 * BL], BF16, kind="ExternalInput")
    b1t_d = nc.dram_tensor("b1t", [128, 16], F32, kind="ExternalInput")
    # vecs: [0:UL]=(b2+bias) u-slice, [UL:UL+128]=ones
    vecs_d = nc.dram_tensor("vecs", [1, UL + 128], BF16, kind="ExternalInput")
    # w1 quarters: w1q[q][p, kk*512+kc*128+c] = w1[kc*128+p, (4q+kk)*128+c]
    w1_d = [
        nc.dram_tensor(f"w1{q}", [128, 2048], BF16, kind="ExternalInput")
        for q in range(4)
    ]
    # w2 halves: w2q[h][p, ww*256+u] = w2[(8h+ww)*128+p, uh*256+u]
    w2_d = [
        nc.dram_tensor(f"w2{q}", [128, 2048], BF16, kind="ExternalInput")
        for q in range(2)
    ]
    # kg quarters: kgq[q][p, ii*256+u] = KG[(8q+ii)*128+p, uh*256+u],
    # KG = g-major (4096, 512) basis kernel with K_g folded in
    kg_d = [
        nc.dram_tensor(f"kg{q}", [128, 2048], BF16, kind="ExternalInput")
        for q in range(4)
    ]
    out_d = nc.dram_tensor("out", [128, 2 * UL], F32, kind="ExternalOutput")

    with ExitStack() as ctx:
        tc = ctx.enter_context(tile.TileContext(nc))
        const = ctx.enter_context(tc.tile_pool(name="const", bufs=1))
        btp = ctx.enter_context(tc.tile_pool(name="btp", bufs=7))
        htp = ctx.enter_context(tc.tile_pool(name="htp", bufs=16))
        wps_pool = ctx.enter_context(
            tc.tile_pool(name="wps", bufs=1, space=bass.MemorySpace.PSUM)
        )
        hps_pool = ctx.enter_context(
            tc.tile_pool(name="hps", bufs=6, space=bass.MemorySpace.PSUM)
        )
        ops_pool = ctx.enter_context(
            tc.tile_pool(name="ops", bufs=1, space=bass.MemorySpace.PSUM)
        )

        # ---- ACT gelu-table preload + PE clock warm-up (no input deps) ----
        gelu_fn = AF.Identity if os.environ.get("TRN_SIM_NOGELU") else AF.Gelu
        warm = const.tile([128, 1], F32, tag="warm")
        nc.gpsimd.memset(warm[:], 0.0)
        warm2 = const.tile([128, 1], F32, tag="warm2")
        nc.scalar.activation(warm2[:], warm[:], gelu_fn)
        wl = const.tile([128, 128], BF16, tag="wl")
        nc.gpsimd.memset(wl[:], 0.0)
        wps = wps_pool.tile([128, 128], F32)
        for _ in range(NWARM):
            nc.tensor.matmul(wps[:], wl[:], wl[:], start=True, stop=True)

        # ---- loads (nc.sync HWDGE => FIFO in emission order) ----
        xt_sb = const.tile([128, 4 * BL], BF16, tag="xt")
        nc.sync.dma_start(xt_sb[:], xt_d[:])
        rt_sb = const.tile([128, 8 * BL], BF16, tag="rt")
        nc.sync.dma_start(rt_sb[:], rt_d[:])
        b1t_sb = const.tile([128, 16], F32, tag="b1t")
        nc.sync.dma_start(b1t_sb[:], b1t_d[:])
        vecs_sb = const.tile([1, UL + 128], BF16, tag="vecs")
        nc.sync.dma_start(vecs_sb[:], vecs_d[:])
        w1_sb = []
        for q in range(4):
            t = const.tile([128, 2048], BF16, tag=f"w1{q}")
            nc.sync.dma_start(t[:], w1_d[q][:])
            w1_sb.append(t)
        w2_sb = []
        for q in range(2):
            t = const.tile([128, 2048], BF16, tag=f"w2{q}")
            nc.sync.dma_start(t[:], w2_d[q][:])
            w2_sb.append(t)
        kg_sb = []
        for q in range(4):
            t = const.tile([128, 2048], BF16, tag=f"kg{q}")
            nc.sync.dma_start(t[:], kg_d[q][:])
            kg_sb.append(t)

        def w1_blk(kc, k):  # lhsT [128 f, 128 h] for f-chunk kc, h-chunk k
            q, kk = divmod(k, 4)
            return w1_sb[q][:, kk * 512 + kc * 128 : kk * 512 + (kc + 1) * 128]

        def w2_chunk(k):  # rhs [128 h, UL] for h-chunk k
            q, ww = divmod(k, 8)
            return w2_sb[q][:, ww * UL : (ww + 1) * UL]

        def kg_chunk(i):  # rhs [128 fg, UL] for contraction chunk i of 32
            q, ii = divmod(i, 8)
            return kg_sb[q][:, ii * UL : (ii + 1) * UL]

        bcv = vecs_sb[0:1, 0:UL]
        ones = vecs_sb[0:1, UL : UL + 128]

        # ---- basis: bt[g] = t0 * r^g, bf16 DVE recurrence ----
        bt = [rt_sb[:, 0 : 4 * BL]]  # g=0 is t0 itself
        r_v = rt_sb[:, 4 * BL : 8 * BL]
        for g in range(1, G):
            c = btp.tile([128, 4 * BL], BF16, tag="bt")
            nc.vector.tensor_mul(c[:], bt[-1][:], r_v)
            bt.append(c[:])

        # ---- MLP1 weight-stationary: hT psum tiles + fused-bias gelu ----
        ht = []
        for k in range(16):
            hps = hps_pool.tile([128, BL], F32)
            for kc in range(4):
                nc.tensor.matmul(
                    hps[:],
                    w1_blk(kc, k),
                    xt_sb[:, kc * BL : (kc + 1) * BL],
                    start=(kc == 0),
                    stop=(kc == 3),
                )
            t = htp.tile([128, BL], BF16, tag="ht")
            nc.scalar.activation(t[:], hps[:], gelu_fn, bias=b1t_sb[:, k : k + 1])
            ht.append(t)

        # ---- accumulation bank: (b2+bias) -> MLP2 -> KAN, 2 batch halves
        # in disjoint column ranges of one PSUM bank ----
        out_ps = ops_pool.tile([128, 2 * UL], F32)
        oh = [out_ps[:, 0:UL], out_ps[:, UL : 2 * UL]]
        for h in range(2):
            nc.tensor.matmul(
                oh[h], ones, bcv, start=True, stop=False, skip_group_check=True
            )
        for k in range(16):
            for h in range(2):
                nc.tensor.matmul(
                    oh[h],
                    ht[k][:, h * 128 : (h + 1) * 128],
                    w2_chunk(k),
                    start=False,
                    stop=False,
                    skip_group_check=True,
                )
        out_sb = const.tile([128, 2 * UL], F32, tag="outsb")

        def kan_mm(i, h, stop):
            g, j = divmod(i, 4)
            nc.tensor.matmul(
                oh[h],
                bt[g][:, j * BL + h * 128 : j * BL + (h + 1) * 128],
                kg_chunk(i),
                start=False,
                stop=stop,
                skip_group_check=True,
            )

        for i in range(24):
            kan_mm(i, 0, False)
            kan_mm(i, 1, False)
        for i in range(24, 32):  # half-0 finishes first -> early store
            kan_mm(i, 0, i == 31)
        nc.vector.tensor_copy(out_sb[:, 0:UL], oh[0])
        nc.sync.dma_start(out_d[:, 0:UL], out_sb[:, 0:UL])
        for i in range(24, 32):
            kan_mm(i, 1, i == 31)
        nc.vector.tensor_copy(out_sb[:, UL : 2 * UL], oh[1])
        nc.sync.dma_start(out_d[:, UL : 2 * UL], out_sb[:, UL : 2 * UL])

    nc.compile()
    return nc


def _host_prep(basis_kernel, mlp_w1, mlp_b1, mlp_w2, mlp_b2, bias):
    """Packing shared across cores (weights)."""
    # w1 quarters: (kc, p, k, c) -> (p, k, kc, c)
    w1r = mlp_w1.reshape(4, 128, 16, 128).transpose(1, 2, 0, 3).reshape(128, 16 * 512)
    w1q = [np.ascontiguousarray(w1r[:, q * 2048 : (q + 1) * 2048]).astype(bf16)
           for q in range(4)]
    # g-major with K_g = exp(-(2g/7)^2) folded in
    gidx = np.arange(G, dtype=np.float64)
    kscale = np.exp(-((2.0 * gidx / 7.0) ** 2)).astype(np.float32)
    KG = (basis_kernel.transpose(1, 0, 2) * kscale[:, None, None]).reshape(G * F, U)
    b1t = np.ascontiguousarray(mlp_b1.reshape(16, 128).T).astype(np.float32)
    return w1q, KG, b1t


def kernel(x, basis_kernel, mlp_w1, mlp_b1, mlp_w2, mlp_b2, bias):
    global _prog_cache
    x = np.asarray(x, dtype=np.float32)
    basis_kernel = np.asarray(basis_kernel, dtype=np.float32)
    mlp_w1 = np.asarray(mlp_w1, dtype=np.float32)
    mlp_b1 = np.asarray(mlp_b1, dtype=np.float32)
    mlp_w2 = np.asarray(mlp_w2, dtype=np.float32)
    mlp_b2 = np.asarray(mlp_b2, dtype=np.float32)
    bias = np.asarray(bias, dtype=np.float32)

    w1q, KG, b1t = _host_prep(basis_kernel, mlp_w1, mlp_b1, mlp_w2, mlp_b2, bias)

    # per-U-shard packs
    ush = []
    for uh in range(RU):
        KGu = KG[:, uh * UL : (uh + 1) * UL].reshape(32, 128, UL)
        kgq = [np.ascontiguousarray(
                   KGu[8 * q : 8 * q + 8].transpose(1, 0, 2).reshape(128, 8 * UL)
               ).astype(bf16) for q in range(4)]
        w2u = mlp_w2[:, uh * UL : (uh + 1) * UL].reshape(16, 128, UL)
        w2q = [np.ascontiguousarray(
                   w2u[8 * q : 8 * q + 8].transpose(1, 0, 2).reshape(128, 8 * UL)
               ).astype(bf16) for q in range(2)]
        vecs = np.zeros((1, UL + 128), bf16)
        vecs[0, :UL] = (mlp_b2 + bias)[uh * UL : (uh + 1) * UL].astype(bf16)
        vecs[0, UL:] = np.ones(128, bf16)
        ush.append((kgq, w2q, vecs))

    # per-batch-shard packs
    bsh = []
    for s in range(RB):
        xs = x[s * BL : (s + 1) * BL].astype(np.float64)  # [256, 512]
        y = xs + 1.0
        t0 = np.exp(-(y * y))
        r = np.exp(4.0 * y / 7.0)

        def pack(a):  # [256,512] -> [128, 1024]: [p, j*256+b] = a[b, j*128+p]
            return np.ascontiguousarray(
                a.reshape(BL, 4, 128).transpose(2, 1, 0).reshape(128, 4 * BL)
            ).astype(bf16)

        xt = pack(xs)
        rt = np.concatenate([pack(t0), pack(r)], axis=1)
        bsh.append((xt, rt))

    in_maps = []
    for c in range(NCORES):
        s, uh = divmod(c, RU)
        xt, rt = bsh[s]
        kgq, w2q, vecs = ush[uh]
        m = {"xt": xt, "rt": rt, "b1t": b1t, "vecs": vecs}
        for q in range(4):
            m[f"w1{q}"] = w1q[q]
            m[f"kg{q}"] = kgq[q]
        for q in range(2):
            m[f"w2{q}"] = w2q[q]
        in_maps.append(m)

    if _prog_cache is None:
        _prog_cache = _build_program()
    nc = _prog_cache

    trace = bool(int(os.environ.get("TRN_KERNEL_TRACE", "0")))
    if trace:
        _install_profile_hook()
    res = run_bass_kernel_spmd(
        nc,
        in_maps,
        core_ids=list(range(NCORES)),
        trace=trace,
    )
    if trace:
        print(f"HW exec time: {res.exec_time_ns} ns")
        kernel.last_results = res

    out = np.empty((B, U), np.float32)
    for c in range(NCORES):
        s, uh = divmod(c, RU)
        oc = res.results[c]["out"]  # [128, 512]
        blk = oc.reshape(128, 2, UL).transpose(1, 0, 2).reshape(BL, UL)
        out[s * BL : (s + 1) * BL, uh * UL : (uh + 1) * UL] = blk
    return out.astype(np.float32)


kernel.last_results = None


def _install_profile_hook():
    """The image lacks antenv.axon_hooks; synthesize it so
    run_bass_kernel_spmd(trace=True) can reach the NTFF profiler in
    libaxon_pjrt.so.  Test-only path (TRN_KERNEL_TRACE=1)."""
    import sys
    import types

    if "antenv.axon_hooks" not in sys.modules:
        mod = types.ModuleType("antenv.axon_hooks")
        mod._hook = None

        def set_axon_ntff_profile_hook(h):
            mod._hook = h

        def get_axon_ntff_profile_hook():
            return mod._hook

        mod.set_axon_ntff_profile_hook = set_axon_ntff_profile_hook
        mod.get_axon_ntff_profile_hook = get_axon_ntff_profile_hook
        sys.modules["antenv.axon_hooks"] = mod
        import antenv

        antenv.axon_hooks = mod
        from trn_agent_boot.trn_boot import _ntff_profile_via_ctypes

        mod.set_axon_ntff_profile_hook(
            _ntff_profile_via_ctypes("/opt/axon/libaxon_pjrt.so")
        )
    import concourse.bass_utils as _bu

    _bu.upload_artifacts = lambda tmpdir: f"local:{tmpdir}"


# revision 9
# speedup vs baseline: 1.0764x; 1.0240x over previous
"""Trainium2 Bass kernel for DenseKANRBF.

Computation (per reference):
    centers c_g = linspace(-1, 1, 8)  (same for every feature)
    basis[b,f,g] = exp(-(x[b,f] - c_g)^2)
    out = einsum('bfg,fgu->bu', basis, basis_kernel)
        + gelu(x @ w1 + b1, exact) @ w2 + b2 + bias

Shapes: B=1024, F=512, G=8, U=512, H=2048 (fp32).

Strategy: HYBRID sharding over the 8 NeuronCores: 4 batch shards x 2
U-shards (rb=4, ru=2).  Each core handles 256 batch rows and 256 output
columns, so per-core DMA drops from 8.6MB (pure data-parallel,
replicated weights) to ~5.8MB, while PE work stays ~17us -- balanced
against the ~16us DMA at 360GB/s.  No cross-core communication; the
host scatters shards and gathers the 4x2 output grid.

Key device-side choices (from trace/cost-model analysis of the 41us
data-parallel baseline, which was pinned at the replicated-weight DMA
floor with a 3.4us PE stall + clock droop before the KAN tail):

  - The uniform grid makes the RBF basis a geometric sequence:
        basis_g = K_g * t0 * r^g,  t0 = exp(-(x+1)^2), r = exp(4(x+1)/7)
    K_g is folded into basis_kernel on the host.  t0/r are computed on
    device (2 Exp ACTs + 2 DVE f32 ops), then 7 bf16 DVE multiplies
    produce the whole basis.  Scalar order: exp-table dummy, r, t0,
    then gelus (one implicit gelu-table load) -- MLP1 is DMA-paced so
    the late gelu start cannot stall the PSUM recycling.
  - Every dma_start costs ~640ns of serialized DIRECT2D descriptor
    generation on the issuing sequencer, so small/latency-insensitive
    loads (b1t, vecs) go on the GpSimd DGE queue while the
    PE-critical stream (xt, w1, w2, kg, out) stays in order on Sync.
  - All tensors are pre-packed/transposed on the host so every matmul
    operand is a contiguous [128, N] slice: xt/basis in f-partition
    packed layout, w1/w2/kg in contraction-partition packed layout.
  - Weight DMAs are split into 512KB chunks (w1 x4, w2 x2, kg x4) so
    matmuls track chunk arrival instead of whole-tensor completion.
  - PE order: warmup (clock ramp) -> MLP1 (64 mm) -> bias+MLP2 (34 mm)
    -> KAN (64 mm), with both batch halves accumulating in disjoint
    column ranges of ONE PSUM bank; half-0's chain stops 8 matmuls
    early so its copy+store overlaps half-1's tail.
"""

import os
from contextlib import ExitStack

import numpy as np
import ml_dtypes

import concourse.bass as bass
import concourse.bacc as bacc
import concourse.mybir as mybir
from concourse import tile
from concourse.bass_utils import run_bass_kernel_spmd

F32 = mybir.dt.float32
BF16 = mybir.dt.bfloat16
AF = mybir.ActivationFunctionType

B, F, G, U, H = 1024, 512, 8, 512, 2048
NCORES = 8
RB, RU = 4, 2  # batch shards x U shards
BL = B // RB  # 256 batch rows per core
UL = U // RU  # 256 output cols per core
NWARM = 20  # PE clock-ramp warm-up matmuls (bridge until w1 chunk 0 lands)

bf16 = ml_dtypes.bfloat16

_prog_cache = None


def _build_program():
    nc = bacc.Bacc("TRN2", target_bir_lowering=False, debug=False, num_devices=NCORES)

    # xt[p, j*256+b] = x[s*256+b, j*128+p]   (f-chunk j, batch b)
    xt_d = nc.dram_tensor("xt", [128, 4 * BL], BF16, kind="ExternalInput")
    # cols 0..15: b1 per-partition; col 16: 4/7; col 17: -1.0
    b1t_d = nc.dram_tensor("b1t", [128, 18], F32, kind="ExternalInput")
    # vecs: [0:UL]=(b2+bias) u-slice, [UL:UL+128]=ones
    vecs_d = nc.dram_tensor("vecs", [1, UL + 128], BF16, kind="ExternalInput")
    # w1 quarters: w1q[q][p, kk*512+kc*128+c] = w1[kc*128+p, (4q+kk)*128+c]
    w1_d = [
        nc.dram_tensor(f"w1{q}", [128, 2048], BF16, kind="ExternalInput")
        for q in range(4)
    ]
    # w2 halves: w2q[h][p, ww*256+u] = w2[(8h+ww)*128+p, uh*256+u]
    w2_d = [
        nc.dram_tensor(f"w2{q}", [128, 2048], BF16, kind="ExternalInput")
        for q in range(2)
    ]
    # kg quarters: kgq[q][p, ii*256+u] = KG[(8q+ii)*128+p, uh*256+u],
    # KG = g-major (4096, 512) basis kernel with K_g folded in
    kg_d = [
        nc.dram_tensor(f"kg{q}", [128, 2048], BF16, kind="ExternalInput")
        for q in range(4)
    ]
    out_d = nc.dram_tensor("out", [128, 2 * UL], F32, kind="ExternalOutput")

    with ExitStack() as ctx:
        tc = ctx.enter_context(tile.TileContext(nc))
        const = ctx.enter_context(tc.tile_pool(name="const", bufs=1))
        btp = ctx.enter_context(tc.tile_pool(name="btp", bufs=7))
        htp = ctx.enter_context(tc.tile_pool(name="htp", bufs=16))
        wps_pool = ctx.enter_context(
            tc.tile_pool(name="wps", bufs=1, space=bass.MemorySpace.PSUM)
        )
        hps_pool = ctx.enter_context(
            tc.tile_pool(name="hps", bufs=6, space=bass.MemorySpace.PSUM)
        )
        ops_pool = ctx.enter_context(
            tc.tile_pool(name="ops", bufs=1, space=bass.MemorySpace.PSUM)
        )

        # ---- ACT exp-table preload + PE clock warm-up (no input deps) ----
        gelu_fn = AF.Identity if os.environ.get("TRN_SIM_NOGELU") else AF.Gelu
        warm = const.tile([128, 1], F32, tag="warm")
        nc.gpsimd.memset(warm[:], 0.0)
        warm2 = const.tile([128, 1], F32, tag="warm2")
        nc.scalar.activation(warm2[:], warm[:], AF.Exp)
        wl = const.tile([128, 128], BF16, tag="wl")
        nc.gpsimd.memset(wl[:], 0.0)
        wr = const.tile([128, 512], BF16, tag="wr")
        nc.gpsimd.memset(wr[:], 0.0)
        wps = wps_pool.tile([128, 512], F32)
        for _ in range(NWARM):
            nc.tensor.matmul(wps[:], wl[:], wr[:], start=True, stop=True)

        # ---- loads: PE-critical stream on sync, small stuff on gpsimd ----
        xt_sb = const.tile([128, 4 * BL], BF16, tag="xt")
        nc.sync.dma_start(xt_sb[:], xt_d[:])
        b1t_sb = const.tile([128, 18], F32, tag="b1t")
        nc.gpsimd.dma_start(b1t_sb[:], b1t_d[:])
        vecs_sb = const.tile([1, UL + 128], BF16, tag="vecs")
        nc.gpsimd.dma_start(vecs_sb[:], vecs_d[:])
        w1_sb = []
        for q in range(4):
            t = const.tile([128, 2048], BF16, tag=f"w1{q}")
            nc.sync.dma_start(t[:], w1_d[q][:])
            w1_sb.append(t)
        w2_sb = []
        for q in range(2):
            t = const.tile([128, 2048], BF16, tag=f"w2{q}")
            nc.sync.dma_start(t[:], w2_d[q][:])
            w2_sb.append(t)
        kg_sb = []
        for q in range(4):
            t = const.tile([128, 2048], BF16, tag=f"kg{q}")
            nc.sync.dma_start(t[:], kg_d[q][:])
            kg_sb.append(t)

        def w1_blk(kc, k):  # lhsT [128 f, 128 h] for f-chunk kc, h-chunk k
            q, kk = divmod(k, 4)
            return w1_sb[q][:, kk * 512 + kc * 128 : kk * 512 + (kc + 1) * 128]

        def w2_chunk(k):  # rhs [128 h, UL] for h-chunk k
            q, ww = divmod(k, 8)
            return w2_sb[q][:, ww * UL : (ww + 1) * UL]

        def kg_chunk(i):  # rhs [128 fg, UL] for contraction chunk i of 32
            q, ii = divmod(i, 8)
            return kg_sb[q][:, ii * UL : (ii + 1) * UL]

        bcv = vecs_sb[0:1, 0:UL]
        ones = vecs_sb[0:1, UL : UL + 128]

        # ---- basis: bt[g] = t0 * r^g ----
        # r = exp((4/7)x + 4/7) straight from xt; t0 = exp(-(x+1)^2) via
        # y = x+1, s = y*y on DVE.  Both Exps write bf16; the recurrence
        # is 7 bf16 DVE multiplies.
        r_sb = const.tile([128, 4 * BL], BF16, tag="r")
        c47 = b1t_sb[:, 16:17]
        cm1 = b1t_sb[:, 17:18]
        nc.scalar.activation(r_sb[:], xt_sb[:], AF.Exp, bias=c47, scale=c47)
        y_sb = const.tile([128, 4 * BL], F32, tag="y")
        nc.vector.tensor_scalar_add(y_sb[:], xt_sb[:], 1.0)
        s_sb = const.tile([128, 4 * BL], F32, tag="s")
        nc.vector.tensor_mul(s_sb[:], y_sb[:], y_sb[:])
        t0_sb = const.tile([128, 4 * BL], BF16, tag="t0")
        nc.scalar.activation(t0_sb[:], s_sb[:], AF.Exp, scale=cm1)
        bt = [t0_sb[:]]
        for g in range(1, G):
            c = btp.tile([128, 4 * BL], BF16, tag="bt")
            nc.vector.tensor_mul(c[:], bt[-1][:], r_sb[:])
            bt.append(c[:])

        # ---- MLP1 weight-stationary: hT psum tiles + fused-bias gelu ----
        ht = []
        for k in range(16):
            hps = hps_pool.tile([128, BL], F32)
            for kc in range(4):
                nc.tensor.matmul(
                    hps[:],
                    w1_blk(kc, k),
                    xt_sb[:, kc * BL : (kc + 1) * BL],
                    start=(kc == 0),
                    stop=(kc == 3),
                )
            t = htp.tile([128, BL], BF16, tag="ht")
            nc.scalar.activation(t[:], hps[:], gelu_fn, bias=b1t_sb[:, k : k + 1])
            ht.append(t)

        # ---- accumulation bank: (b2+bias) -> MLP2 -> KAN, 2 batch halves
        # in disjoint column ranges of one PSUM bank ----
        out_ps = ops_pool.tile([128, 2 * UL], F32)
        oh = [out_ps[:, 0:UL], out_ps[:, UL : 2 * UL]]
        for h in range(2):
            nc.tensor.matmul(
                oh[h], ones, bcv, start=True, stop=False, skip_group_check=True
            )
        for k in range(16):
            for h in range(2):
                nc.tensor.matmul(
                    oh[h],
                    ht[k][:, h * 128 : (h + 1) * 128],
                    w2_chunk(k),
                    start=False,
                    stop=False,
                    skip_group_check=True,
                )
        out_sb = const.tile([128, 2 * UL], F32, tag="outsb")

        def kan_mm(i, h, stop):
            g, j = divmod(i, 4)
            nc.tensor.matmul(
                oh[h],
                bt[g][:, j * BL + h * 128 : j * BL + (h + 1) * 128],
                kg_chunk(i),
                start=False,
                stop=stop,
                skip_group_check=True,
            )

        for i in range(24):
            kan_mm(i, 0, False)
            kan_mm(i, 1, False)
        for i in range(24, 32):  # half-0 finishes first -> early store
            kan_mm(i, 0, i == 31)
        nc.vector.tensor_copy(out_sb[:, 0:UL], oh[0])
        nc.sync.dma_start(out_d[:, 0:UL], out_sb[:, 0:UL])
        for i in range(24, 32):
            kan_mm(i, 1, i == 31)
        nc.vector.tensor_copy(out_sb[:, UL : 2 * UL], oh[1])
        nc.sync.dma_start(out_d[:, UL : 2 * UL], out_sb[:, UL : 2 * UL])

    nc.compile()
    return nc


def _host_prep(basis_kernel, mlp_w1, mlp_b1, mlp_w2, mlp_b2, bias):
    """Packing shared across cores (weights)."""
    # w1 quarters: (kc, p, k, c) -> (p, k, kc, c)
    w1r = mlp_w1.reshape(4, 128, 16, 128).transpose(1, 2, 0, 3).reshape(128, 16 * 512)
    w1q = [np.ascontiguousarray(w1r[:, q * 2048 : (q + 1) * 2048]).astype(bf16)
           for q in range(4)]
    # g-major with K_g = exp(-(2g/7)^2) folded in
    gidx = np.arange(G, dtype=np.float64)
    kscale = np.exp(-((2.0 * gidx / 7.0) ** 2)).astype(np.float32)
    KG = (basis_kernel.transpose(1, 0, 2) * kscale[:, None, None]).reshape(G * F, U)
    b1t = np.zeros((128, 18), np.float32)
    b1t[:, :16] = mlp_b1.reshape(16, 128).T
    b1t[:, 16] = 4.0 / 7.0
    b1t[:, 17] = -1.0
    return w1q, KG, b1t


def kernel(x, basis_kernel, mlp_w1, mlp_b1, mlp_w2, mlp_b2, bias):
    global _prog_cache
    x = np.asarray(x, dtype=np.float32)
    basis_kernel = np.asarray(basis_kernel, dtype=np.float32)
    mlp_w1 = np.asarray(mlp_w1, dtype=np.float32)
    mlp_b1 = np.asarray(mlp_b1, dtype=np.float32)
    mlp_w2 = np.asarray(mlp_w2, dtype=np.float32)
    mlp_b2 = np.asarray(mlp_b2, dtype=np.float32)
    bias = np.asarray(bias, dtype=np.float32)

    w1q, KG, b1t = _host_prep(basis_kernel, mlp_w1, mlp_b1, mlp_w2, mlp_b2, bias)

    # per-U-shard packs
    ush = []
    for uh in range(RU):
        KGu = KG[:, uh * UL : (uh + 1) * UL].reshape(32, 128, UL)
        kgq = [np.ascontiguousarray(
                   KGu[8 * q : 8 * q + 8].transpose(1, 0, 2).reshape(128, 8 * UL)
               ).astype(bf16) for q in range(4)]
        w2u = mlp_w2[:, uh * UL : (uh + 1) * UL].reshape(16, 128, UL)
        w2q = [np.ascontiguousarray(
                   w2u[8 * q : 8 * q + 8].transpose(1, 0, 2).reshape(128, 8 * UL)
               ).astype(bf16) for q in range(2)]
        vecs = np.zeros((1, UL + 128), bf16)
        vecs[0, :UL] = (mlp_b2 + bias)[uh * UL : (uh + 1) * UL].astype(bf16)
        vecs[0, UL:] = np.ones(128, bf16)
        ush.append((kgq, w2q, vecs))

    # per-batch-shard packs
    bsh = []
    for s in range(RB):
        xs = x[s * BL : (s + 1) * BL]  # [256, 512]
        # [256,512] -> [128, 1024]: [p, j*256+b] = xs[b, j*128+p]
        xt = np.ascontiguousarray(
            xs.reshape(BL, 4, 128).transpose(2, 1, 0).reshape(128, 4 * BL)
        ).astype(bf16)
        bsh.append(xt)

    in_maps = []
    for c in range(NCORES):
        s, uh = divmod(c, RU)
        xt = bsh[s]
        kgq, w2q, vecs = ush[uh]
        m = {"xt": xt, "b1t": b1t, "vecs": vecs}
        for q in range(4):
            m[f"w1{q}"] = w1q[q]
            m[f"kg{q}"] = kgq[q]
        for q in range(2):
            m[f"w2{q}"] = w2q[q]
        in_maps.append(m)

    if _prog_cache is None:
        _prog_cache = _build_program()
    nc = _prog_cache

    trace = bool(int(os.environ.get("TRN_KERNEL_TRACE", "0")))
    if trace:
        _install_profile_hook()
    res = run_bass_kernel_spmd(
        nc,
        in_maps,
        core_ids=list(range(NCORES)),
        trace=trace,
    )
    if trace:
        print(f"HW exec time: {res.exec_time_ns} ns")
        kernel.last_results = res

    out = np.empty((B, U), np.float32)
    for c in range(NCORES):
        s, uh = divmod(c, RU)
        oc = res.results[c]["out"]  # [128, 512]
        blk = oc.reshape(128, 2, UL).transpose(1, 0, 2).reshape(BL, UL)
        out[s * BL : (s + 1) * BL, uh * UL : (uh + 1) * UL] = blk
    return out.astype(np.float32)


kernel.last_results = None


def _install_profile_hook():
    """The image lacks antenv.axon_hooks; synthesize it so
    run_bass_kernel_spmd(trace=True) can reach the NTFF profiler in
    libaxon_pjrt.so.  Test-only path (TRN_KERNEL_TRACE=1)."""
    import sys
    import types

    if "antenv.axon_hooks" not in sys.modules:
        mod = types.ModuleType("antenv.axon_hooks")
        mod._hook = None

        def set_axon_ntff_profile_hook(h):
            mod._hook = h

        def get_axon_ntff_profile_hook():
            return mod._hook

        mod.set_axon_ntff_profile_hook = set_axon_ntff_profile_hook
        mod.get_axon_ntff_profile_hook = get_axon_ntff_profile_hook
        sys.modules["antenv.axon_hooks"] = mod
        import antenv

        antenv.axon_hooks = mod
        from trn_agent_boot.trn_boot import _ntff_profile_via_ctypes

        mod.set_axon_ntff_profile_hook(
            _ntff_profile_via_ctypes("/opt/axon/libaxon_pjrt.so")
        )
    import concourse.bass_utils as _bu

    _bu.upload_artifacts = lambda tmpdir: f"local:{tmpdir}"


# revision 12
# speedup vs baseline: 1.1119x; 1.0330x over previous
"""Trainium2 Bass kernel for DenseKANRBF.

Computation (per reference):
    centers c_g = linspace(-1, 1, 8)  (same for every feature)
    basis[b,f,g] = exp(-(x[b,f] - c_g)^2)
    out = einsum('bfg,fgu->bu', basis, basis_kernel)
        + gelu(x @ w1 + b1, exact) @ w2 + b2 + bias

Shapes: B=1024, F=512, G=8, U=512, H=2048 (fp32).

Strategy: HYBRID sharding over the 8 NeuronCores: 4 batch shards x 2
U-shards (rb=4, ru=2).  Each core handles 256 batch rows and 256 output
columns, so per-core DMA drops from 8.6MB (pure data-parallel,
replicated weights) to ~5.8MB, while PE work stays ~17us -- balanced
against the ~16us DMA at 360GB/s.  No cross-core communication; the
host scatters shards and gathers the 4x2 output grid.

Key device-side choices (from trace/cost-model analysis of the 41us
data-parallel baseline, which was pinned at the replicated-weight DMA
floor with a 3.4us PE stall + clock droop before the KAN tail):

  - The uniform grid makes the RBF basis a geometric sequence:
        basis_g = K_g * t0 * r^g,  t0 = exp(-(x+1)^2), r = exp(4(x+1)/7)
    K_g is folded into basis_kernel on the host.  t0/r are computed on
    device (2 Exp ACTs + 2 DVE f32 ops), then 7 bf16 DVE multiplies
    produce the whole basis.  Scalar order: exp-table dummy, r, t0,
    then gelus (one implicit gelu-table load) -- MLP1 is DMA-paced so
    the late gelu start cannot stall the PSUM recycling.
  - Every dma_start costs ~640ns of serialized DIRECT2D descriptor
    generation on the issuing sequencer, so small/latency-insensitive
    loads (b1t, vecs) go on the GpSimd DGE queue while the
    PE-critical stream (xt, w1, w2, kg, out) stays in order on Sync.
  - All tensors are pre-packed/transposed on the host so every matmul
    operand is a contiguous [128, N] slice: xt/basis in f-partition
    packed layout, w1/w2/kg in contraction-partition packed layout.
  - Weight DMAs are split into 512KB chunks (w1 x4, w2 x2, kg x4) so
    matmuls track chunk arrival instead of whole-tensor completion.
  - PE order: warmup (clock ramp) -> MLP1 (64 mm) -> bias+MLP2 (34 mm)
    -> KAN (64 mm), with both batch halves accumulating in disjoint
    column ranges of ONE PSUM bank; half-0's chain stops 8 matmuls
    early so its copy+store overlaps half-1's tail.
"""

import os
from contextlib import ExitStack

import numpy as np
import ml_dtypes

import concourse.bass as bass
import concourse.bacc as bacc
import concourse.mybir as mybir
from concourse import tile
from concourse.bass_utils import run_bass_kernel_spmd

F32 = mybir.dt.float32
BF16 = mybir.dt.bfloat16
AF = mybir.ActivationFunctionType

B, F, G, U, H = 1024, 512, 8, 512, 2048
NCORES = 8
RB, RU = 4, 2  # batch shards x U shards
BL = B // RB  # 256 batch rows per core
UL = U // RU  # 256 output cols per core
NWARM = 10  # PE clock-ramp warm-up matmuls (bridge until w1 chunk 0 lands)

bf16 = ml_dtypes.bfloat16

_prog_cache = None


def _build_program():
    nc = bacc.Bacc("TRN2", target_bir_lowering=False, debug=False, num_devices=NCORES)

    # xt[p, j*256+b] = x[s*256+b, j*128+p]   (f-chunk j, batch b)
    xt_d = nc.dram_tensor("xt", [128, 4 * BL], BF16, kind="ExternalInput")
    # cols 0..15: b1 per-partition; col 16: 4/7; col 17: -1.0
    b1t_d = nc.dram_tensor("b1t", [128, 18], F32, kind="ExternalInput")
    # vecs: [0:UL]=(b2+bias) u-slice, [UL:UL+128]=ones
    vecs_d = nc.dram_tensor("vecs", [1, UL + 128], BF16, kind="ExternalInput")
    # w1 quarters: w1q[q][p, kk*512+kc*128+c] = w1[kc*128+p, (4q+kk)*128+c]
    w1_d = [
        nc.dram_tensor(f"w1{q}", [128, 2048], BF16, kind="ExternalInput")
        for q in range(4)
    ]
    # w2 halves: w2q[h][p, ww*256+u] = w2[(8h+ww)*128+p, uh*256+u]
    w2_d = [
        nc.dram_tensor(f"w2{q}", [128, 2048], BF16, kind="ExternalInput")
        for q in range(2)
    ]
    # kg quarters: kgq[q][p, ii*256+u] = KG[(8q+ii)*128+p, uh*256+u],
    # KG = g-major (4096, 512) basis kernel with K_g folded in
    kg_d = [
        nc.dram_tensor(f"kg{q}", [128, 2048], BF16, kind="ExternalInput")
        for q in range(4)
    ]
    out_d = nc.dram_tensor("out", [128, 2 * UL], BF16, kind="ExternalOutput")

    with ExitStack() as ctx:
        tc = ctx.enter_context(tile.TileContext(nc))
        const = ctx.enter_context(tc.tile_pool(name="const", bufs=1))
        btp = ctx.enter_context(tc.tile_pool(name="btp", bufs=7))
        htp = ctx.enter_context(tc.tile_pool(name="htp", bufs=16))
        # ---- ACT exp-table preload + PE clock warm-up (no input deps) ----
        # warm-up PSUM bank is scoped so its bank is free for the two
        # output accumulators (separate banks => half-0's store copy
        # cannot stall half-1's matmuls on a bank conflict)
        gelu_fn = AF.Identity if os.environ.get("TRN_SIM_NOGELU") else AF.Gelu
        warm = const.tile([128, 1], F32, tag="warm")
        nc.gpsimd.memset(warm[:], 0.0)
        warm2 = const.tile([128, 1], F32, tag="warm2")
        nc.scalar.activation(warm2[:], warm[:], AF.Exp)
        wl = const.tile([128, 128], BF16, tag="wl")
        nc.vector.memset(wl[:], 0.0)
        wr = const.tile([128, 512], BF16, tag="wr")
        nc.vector.memset(wr[:], 0.0)
        with tc.tile_pool(name="wps", bufs=1, space=bass.MemorySpace.PSUM) as wpool:
            wps = wpool.tile([128, 512], F32)
            for _ in range(NWARM):
                nc.tensor.matmul(wps[:], wl[:], wr[:], start=True, stop=True)
        hps_pool = ctx.enter_context(
            tc.tile_pool(name="hps", bufs=6, space=bass.MemorySpace.PSUM)
        )
        ops_pool = ctx.enter_context(
            tc.tile_pool(name="ops", bufs=1, space=bass.MemorySpace.PSUM)
        )

        # ---- loads: PE-critical stream on sync, small stuff on gpsimd ----
        xt_sb = const.tile([128, 4 * BL], BF16, tag="xt")
        nc.sync.dma_start(xt_sb[:], xt_d[:])
        b1t_sb = const.tile([128, 18], F32, tag="b1t")
        nc.gpsimd.dma_start(b1t_sb[:], b1t_d[:])
        vecs_sb = const.tile([1, UL + 128], BF16, tag="vecs")
        nc.gpsimd.dma_start(vecs_sb[:], vecs_d[:])
        w1_sb = []
        for q in range(4):
            t = const.tile([128, 2048], BF16, tag=f"w1{q}")
            nc.sync.dma_start(t[:], w1_d[q][:])
            w1_sb.append(t)
        w2_sb = []
        for q in range(2):
            t = const.tile([128, 2048], BF16, tag=f"w2{q}")
            nc.sync.dma_start(t[:], w2_d[q][:])
            w2_sb.append(t)
        kg_sb = []
        for q in range(4):
            t = const.tile([128, 2048], BF16, tag=f"kg{q}")
            nc.sync.dma_start(t[:], kg_d[q][:])
            kg_sb.append(t)

        def w1_blk(kc, k):  # lhsT [128 f, 128 h] for f-chunk kc, h-chunk k
            q, kk = divmod(k, 4)
            return w1_sb[q][:, kk * 512 + kc * 128 : kk * 512 + (kc + 1) * 128]

        def w2_chunk(k):  # rhs [128 h, UL] for h-chunk k
            q, ww = divmod(k, 8)
            return w2_sb[q][:, ww * UL : (ww + 1) * UL]

        def kg_chunk(i):  # rhs [128 fg, UL] for contraction chunk i of 32
            q, ii = divmod(i, 8)
            return kg_sb[q][:, ii * UL : (ii + 1) * UL]

        bcv = vecs_sb[0:1, 0:UL]
        ones = vecs_sb[0:1, UL : UL + 128]

        # ---- basis: bt[g] = t0 * r^g ----
        # r = exp((4/7)x + 4/7) straight from xt; t0 = exp(-(x+1)^2) via
        # y = x+1, s = y*y on DVE.  Both Exps write bf16; the recurrence
        # is 7 bf16 DVE multiplies.
        r_sb = const.tile([128, 4 * BL], BF16, tag="r")
        c47 = b1t_sb[:, 16:17]
        cm1 = b1t_sb[:, 17:18]
        nc.scalar.activation(r_sb[:], xt_sb[:], AF.Exp, bias=c47, scale=c47)
        y_sb = const.tile([128, 4 * BL], F32, tag="y")
        nc.vector.tensor_scalar_add(y_sb[:], xt_sb[:], 1.0)
        s_sb = const.tile([128, 4 * BL], F32, tag="s")
        nc.vector.tensor_mul(s_sb[:], y_sb[:], y_sb[:])
        t0_sb = const.tile([128, 4 * BL], BF16, tag="t0")
        nc.scalar.activation(t0_sb[:], s_sb[:], AF.Exp, scale=cm1)
        bt = [t0_sb[:]]
        for g in range(1, G):
            c = btp.tile([128, 4 * BL], BF16, tag="bt")
            nc.vector.tensor_mul(c[:], bt[-1][:], r_sb[:])
            bt.append(c[:])

        # ---- MLP1 weight-stationary: hT psum tiles + fused-bias gelu ----
        ht = []
        for k in range(16):
            hps = hps_pool.tile([128, BL], F32)
            for kc in range(4):
                nc.tensor.matmul(
                    hps[:],
                    w1_blk(kc, k),
                    xt_sb[:, kc * BL : (kc + 1) * BL],
                    start=(kc == 0),
                    stop=(kc == 3),
                )
            t = htp.tile([128, BL], BF16, tag="ht")
            nc.scalar.activation(t[:], hps[:], gelu_fn, bias=b1t_sb[:, k : k + 1])
            ht.append(t)

        # ---- accumulation bank: (b2+bias) -> MLP2 -> KAN, 2 batch halves
        # in disjoint column ranges of one PSUM bank ----
        oh0 = ops_pool.tile([128, UL], F32)
        oh1 = ops_pool.tile([128, UL], F32)
        oh = [oh0, oh1]
        for h in range(2):
            nc.tensor.matmul(
                oh[h][:], ones, bcv, start=True, stop=False, skip_group_check=True
            )
        for k in range(16):
            for h in range(2):
                nc.tensor.matmul(
                    oh[h][:],
                    ht[k][:, h * 128 : (h + 1) * 128],
                    w2_chunk(k),
                    start=False,
                    stop=False,
                    skip_group_check=True,
                )
        out_sb = const.tile([128, 2 * UL], BF16, tag="outsb")

        def kan_mm(i, h, stop):
            g, j = divmod(i, 4)
            nc.tensor.matmul(
                oh[h][:],
                bt[g][:, j * BL + h * 128 : j * BL + (h + 1) * 128],
                kg_chunk(i),
                start=False,
                stop=stop,
                skip_group_check=True,
            )

        for i in range(24):
            kan_mm(i, 0, False)
            kan_mm(i, 1, False)
        for i in range(24, 32):  # half-0 finishes first -> early store
            kan_mm(i, 0, i == 31)
        nc.vector.tensor_copy(out_sb[:, 0:UL], oh[0][:])
        nc.sync.dma_start(out_d[:, 0:UL], out_sb[:, 0:UL])
        for i in range(24, 32):
            kan_mm(i, 1, i == 31)
        nc.vector.tensor_copy(out_sb[:, UL : 2 * UL], oh[1][:])
        nc.sync.dma_start(out_d[:, UL : 2 * UL], out_sb[:, UL : 2 * UL])

    nc.compile()
    return nc


def _host_prep(basis_kernel, mlp_w1, mlp_b1, mlp_w2, mlp_b2, bias):
    """Packing shared across cores (weights)."""
    # w1 quarters: (kc, p, k, c) -> (p, k, kc, c)
    w1r = mlp_w1.reshape(4, 128, 16, 128).transpose(1, 2, 0, 3).reshape(128, 16 * 512)
    w1q = [np.ascontiguousarray(w1r[:, q * 2048 : (q + 1) * 2048]).astype(bf16)
           for q in range(4)]
    # g-major with K_g = exp(-(2g/7)^2) folded in
    gidx = np.arange(G, dtype=np.float64)
    kscale = np.exp(-((2.0 * gidx / 7.0) ** 2)).astype(np.float32)
    KG = (basis_kernel.transpose(1, 0, 2) * kscale[:, None, None]).reshape(G * F, U)
    b1t = np.zeros((128, 18), np.float32)
    b1t[:, :16] = mlp_b1.reshape(16, 128).T
    b1t[:, 16] = 4.0 / 7.0
    b1t[:, 17] = -1.0
    return w1q, KG, b1t


def kernel(x, basis_kernel, mlp_w1, mlp_b1, mlp_w2, mlp_b2, bias):
    global _prog_cache
    x = np.asarray(x, dtype=np.float32)
    basis_kernel = np.asarray(basis_kernel, dtype=np.float32)
    mlp_w1 = np.asarray(mlp_w1, dtype=np.float32)
    mlp_b1 = np.asarray(mlp_b1, dtype=np.float32)
    mlp_w2 = np.asarray(mlp_w2, dtype=np.float32)
    mlp_b2 = np.asarray(mlp_b2, dtype=np.float32)
    bias = np.asarray(bias, dtype=np.float32)

    w1q, KG, b1t = _host_prep(basis_kernel, mlp_w1, mlp_b1, mlp_w2, mlp_b2, bias)

    # per-U-shard packs
    ush = []
    for uh in range(RU):
        KGu = KG[:, uh * UL : (uh + 1) * UL].reshape(32, 128, UL)
        kgq = [np.ascontiguousarray(
                   KGu[8 * q : 8 * q + 8].transpose(1, 0, 2).reshape(128, 8 * UL)
               ).astype(bf16) for q in range(4)]
        w2u = mlp_w2[:, uh * UL : (uh + 1) * UL].reshape(16, 128, UL)
        w2q = [np.ascontiguousarray(
                   w2u[8 * q : 8 * q + 8].transpose(1, 0, 2).reshape(128, 8 * UL)
               ).astype(bf16) for q in range(2)]
        vecs = np.zeros((1, UL + 128), bf16)
        vecs[0, :UL] = (mlp_b2 + bias)[uh * UL : (uh + 1) * UL].astype(bf16)
        vecs[0, UL:] = np.ones(128, bf16)
        ush.append((kgq, w2q, vecs))

    # per-batch-shard packs
    bsh = []
    for s in range(RB):
        xs = x[s * BL : (s + 1) * BL]  # [256, 512]
        # [256,512] -> [128, 1024]: [p, j*256+b] = xs[b, j*128+p]
        xt = np.ascontiguousarray(
            xs.reshape(BL, 4, 128).transpose(2, 1, 0).reshape(128, 4 * BL)
        ).astype(bf16)
        bsh.append(xt)

    in_maps = []
    for c in range(NCORES):
        s, uh = divmod(c, RU)
        xt = bsh[s]
        kgq, w2q, vecs = ush[uh]
        m = {"xt": xt, "b1t": b1t, "vecs": vecs}
        for q in range(4):
            m[f"w1{q}"] = w1q[q]
            m[f"kg{q}"] = kgq[q]
        for q in range(2):
            m[f"w2{q}"] = w2q[q]
        in_maps.append(m)

    if _prog_cache is None:
        _prog_cache = _build_program()
    nc = _prog_cache

    trace = bool(int(os.environ.get("TRN_KERNEL_TRACE", "0")))
    if trace:
        _install_profile_hook()
    res = run_bass_kernel_spmd(
        nc,
        in_maps,
        core_ids=list(range(NCORES)),
        trace=trace,
    )
    if trace:
        print(f"HW exec time: {res.exec_time_ns} ns")
        kernel.last_results = res

    out = np.empty((B, U), np.float32)
    for c in range(NCORES):
        s, uh = divmod(c, RU)
        oc = np.asarray(res.results[c]["out"]).astype(np.float32)  # [128, 512]
        blk = oc.reshape(128, 2, UL).transpose(1, 0, 2).reshape(BL, UL)
        out[s * BL : (s + 1) * BL, uh * UL : (uh + 1) * UL] = blk
    return out.astype(np.float32)


kernel.last_results = None


def _install_profile_hook():
    """The image lacks antenv.axon_hooks; synthesize it so
    run_bass_kernel_spmd(trace=True) can reach the NTFF profiler in
    libaxon_pjrt.so.  Test-only path (TRN_KERNEL_TRACE=1)."""
    import sys
    import types

    if "antenv.axon_hooks" not in sys.modules:
        mod = types.ModuleType("antenv.axon_hooks")
        mod._hook = None

        def set_axon_ntff_profile_hook(h):
            mod._hook = h

        def get_axon_ntff_profile_hook():
            return mod._hook

        mod.set_axon_ntff_profile_hook = set_axon_ntff_profile_hook
        mod.get_axon_ntff_profile_hook = get_axon_ntff_profile_hook
        sys.modules["antenv.axon_hooks"] = mod
        import antenv

        antenv.axon_hooks = mod
        from trn_agent_boot.trn_boot import _ntff_profile_via_ctypes

        mod.set_axon_ntff_profile_hook(
            _ntff_profile_via_ctypes("/opt/axon/libaxon_pjrt.so")
        )
    import concourse.bass_utils as _bu

    _bu.upload_artifacts = lambda tmpdir: f"local:{tmpdir}"
